# revision 63
# baseline (speedup 1.0000x reference)
"""EntityAwareAttention Trainium2 kernel.

Problem (per batch b of B=2048):
    hid_e{1,2} = hidden[b, e{1,2}_idx[b]]                       # [600]
    e{1,2}_type = softmax(hid_e @ LT.T) @ LT                    # [600], T=3
    u1 = concat(hidden, pos1, pos2) @ W_hid.T                   # [128, 50]
    u2 = concat(hid_e1, e1_type, hid_e2, e2_type) @ W_ent.T     # [50]
    u = tanh(u1 + u2); scores = u @ v; alpha = softmax(scores)  # [128]
    z = alpha @ hidden[b]                                       # [600]

Sharding: pure data parallel over batch, 8 cores x 256 batches, weights
replicated, host concat of per-core outputs.

Kernel layout strategy (per core):
  - hidden+pos loaded as bf16 via SWDGE cast-DMA into [128 tok, 32 batch, 700]
    tiles (700 features, chunked 5x128 + 60; no padding).
  - u1 needs feature-on-partition layout -> 6 PE transposes per batch,
    evacuated PSUM->SBUF by one split copy (DVE+ACT).
  - u1 matmuls: N=512 (4 batches per group), bf16 in / f32 PSUM accum.
  - u2 add fused into the tanh via ScalarE activation per-partition bias
    (one tanh call per batch: bias = u2[:, b]).
  - scores: one matmul per 4-batch group with v as the 1-column stationary:
    out = v.T @ uT -> [1, 512], written at PSUM partition offset g so the
    round's scores land as [8, 4*128].
  - softmax batched over the [8 group, 4 batch, 128 tok] layout (free-axis
    reduce); alpha normalized in f32, transposed to [128 tok, 32 batch].
  - z: per-batch pair of matmuls with the alpha column as the 1-column
    stationary: out = alphaT[:,b].T @ hp[:,b,:] -> [1, 512] + [1, 88] at
    PSUM partition offset b. One [32, 600] DVE evac + HWDGE store.
  - entity path per 128-batch superbatch: indirect-DMA row gather,
    latent-type softmax via PE transposes, u2 = 20 accumulated matmuls
    (f32 result kept for the tanh bias).
"""

import numpy as np

B, L, H2, PP, A, T = 2048, 128, 600, 50, 50, 3
NCORES = 8
BC = B // NCORES  # 256 batches per core
SB = 128          # superbatch for the entity/u2 pipeline
ROUND = 32        # batches per softmax/z round
GROUP = 4         # batches per u1 matmul group (N = 4*128 = 512)
NG = ROUND // GROUP  # 8 groups per round
F = H2 + 2 * PP   # 700 concat features
FPAD = 768        # padded to 6 x 128
NCH = 6           # feature chunks of 128
EPAD = 640        # 600-dim entity vectors padded to 5 x 128
ECH = 5

_CACHE = {}


def _build_bass(skip=()):
    import os
    skip = set(skip) or set(
        x for x in os.environ.get("KBENCH_SKIP", "").split(",") if x
    )
    import concourse.bass as bass
    import concourse.bacc as bacc
    import concourse.tile as tile
    from concourse import mybir
    from concourse.masks import make_identity

    f32 = mybir.dt.float32
    bf16 = mybir.dt.bfloat16
    fp8 = mybir.dt.float8e4
    i32 = mybir.dt.int32
    AF = mybir.ActivationFunctionType
    AX = mybir.AxisListType
    DR = mybir.MatmulPerfMode.DoubleRow

    nc = bacc.Bacc("TRN2", debug=False, target_bir_lowering=False)

    hid_d = nc.dram_tensor("hidden", [BC, L, H2], f32, kind="ExternalInput").ap()
    pos1_d = nc.dram_tensor("pos1", [BC, L, PP], f32, kind="ExternalInput").ap()
    pos2_d = nc.dram_tensor("pos2", [BC, L, PP], f32, kind="ExternalInput").ap()
    e1r_d = nc.dram_tensor("e1rows", [BC, 1], i32, kind="ExternalInput").ap()
    e2r_d = nc.dram_tensor("e2rows", [BC, 1], i32, kind="ExternalInput").ap()
    whid_d = nc.dram_tensor("w_hid", [A, F], f32, kind="ExternalInput").ap()
    went_d = nc.dram_tensor("w_ent", [A, 4 * H2], f32, kind="ExternalInput").ap()
    lt_d = nc.dram_tensor("latent", [T, H2], f32, kind="ExternalInput").ap()
    v_d = nc.dram_tensor("v", [A, 1], f32, kind="ExternalInput").ap()
    z_d = nc.dram_tensor("z", [BC, H2], bf16, kind="ExternalOutput").ap()

    hid_flat = hid_d.rearrange("b l d -> (b l) d")

    with tile.TileContext(nc) as tc:
        with (
            tc.tile_pool(name="const", bufs=1) as const,
            tc.tile_pool(name="hp_pool", bufs=3) as hp_pool,
            tc.tile_pool(name="ht_pool", bufs=3) as ht_pool,
            tc.tile_pool(name="u_pool", bufs=3) as u_pool,
            tc.tile_pool(name="ent_pool", bufs=2) as ent_pool,
            tc.tile_pool(name="small", bufs=4) as small,
            tc.tile_pool(name="zs_pool", bufs=8) as zs_pool,
            tc.tile_pool(name="ps_tp", bufs=2, space="PSUM") as ps_tp,
            tc.tile_pool(name="ps_u1", bufs=2, space="PSUM") as ps_u1,
            tc.tile_pool(name="ps_sc", bufs=1, space="PSUM") as ps_sc,
            tc.tile_pool(name="ps_z", bufs=2, space="PSUM") as ps_z,
            tc.tile_pool(name="ps_misc", bufs=1, space="PSUM") as ps_misc,
        ):
            # ---------------- one-time constants ----------------
            id_f32 = const.tile([128, 128], f32)
            make_identity(nc, id_f32[:, :])
            id_bf = const.tile([128, 128], bf16)
            nc.vector.tensor_copy(id_bf[:, :], id_f32[:, :])

            # W_hid -> transposed fp8 chunk-pairs [128, 3, 2, 64] for the
            # DoubleRow u1 matmuls (K=256 per pass).
            # (setup-only scratch buffers borrow hp_pool slots: they are
            # dead after the constants are built, so the first hp tiles
            # simply recycle them, keeping SBUF headroom for 3 hp buffers)
            # feature chunks: 5x128 at c*128 plus a final chunk at 572:700
            # that overlaps chunk 4 (so hp needs no zero padding); the
            # overlapped weight rows 0:68 of the last chunk are zeroed.
            CH_OFF = [0, 128, 256, 384, 512, F - 128]
            whid_sb = hp_pool.tile([64, FPAD], f32, tag="hp")
            nc.gpsimd.memset(whid_sb[:, :], 0.0)
            nc.sync.dma_start(out=whid_sb[0:A, 0:F], in_=whid_d)
            whT_ps = ps_u1.tile([128, NCH, 64], f32, tag="u1like")
            for c in range(NCH):
                nc.tensor.transpose(
                    whT_ps[:, c, :], whid_sb[:, CH_OFF[c]:CH_OFF[c] + 128],
                    id_f32[0:64, 0:64],
                )
            whidT8 = const.tile([128, NCH // 2, 2, 64], fp8)
            nc.vector.tensor_copy(
                whidT8[:, :, :, :],
                whT_ps[:, :, :].rearrange("p (cp o) a -> p cp o a", o=2),
            )
            OVL = 128 - (F - CH_OFF[4] - 128)  # overlapped rows of chunk 5
            nc.vector.memset(whidT8[0:OVL, 2, 1, :], 0.0)

            # W_ent -> padded [50, 4*640] then transposed bf16 [128, 20, 50]
            went_sb = hp_pool.tile([A, 4 * EPAD], f32, tag="hp")
            nc.gpsimd.memset(went_sb[:, :], 0.0)
            nc.sync.dma_start(
                out=went_sb[:, :].rearrange("a (q d) -> a q d", q=4)[:, :, 0:H2],
                in_=went_d.rearrange("a (q d) -> a q d", q=4),
            )
            wentT = const.tile([128, 4 * ECH, A], bf16)
            for quarter in range(4):
                weT_ps = ps_u1.tile([128, ECH, 64], f32, tag="u1like")
                for cc in range(ECH):
                    c = quarter * ECH + cc
                    nc.tensor.transpose(
                        weT_ps[:, cc, 0:A],
                        went_sb[:, c * 128:(c + 1) * 128],
                        id_f32[0:A, 0:A],
                    )
                nc.vector.tensor_copy(
                    wentT[:, quarter * ECH:(quarter + 1) * ECH, :],
                    weT_ps[:, :, 0:A],
                )

            # latent_types: padded f32 [3, 640], bf16 copy, transposed chunks
            lt_sb = const.tile([T, EPAD], f32)
            nc.gpsimd.memset(lt_sb[:, :], 0.0)
            nc.sync.dma_start(out=lt_sb[:, 0:H2], in_=lt_d)
            lt16 = const.tile([T, H2], bf16)
            nc.gpsimd.dma_start(out=lt16[:, :], in_=lt_d)
            ltT_ps = ps_u1.tile([128, ECH, 4], f32, tag="u1like")
            for c in range(ECH):
                nc.tensor.transpose(
                    ltT_ps[:, c, 0:T], lt_sb[:, c * 128:(c + 1) * 128],
                    id_f32[0:T, 0:T],
                )
            ltT = const.tile([128, ECH, T], bf16)
            nc.vector.tensor_copy(ltT[:, :, :], ltT_ps[:, :, 0:T])

            v16 = const.tile([A, 1], bf16)
            nc.gpsimd.dma_start(out=v16[:, :], in_=v_d)
            ones_col = const.tile([128, 1], bf16)
            nc.vector.memset(ones_col[:, :], 1.0)

            def entity_gather(s):
                """Row gathers for superbatch s; issued for all superbatches
                up front so the indirect DMAs are in flight early."""
                ents = []
                for rows_d in (e1r_d, e2r_d):
                    rows = ent_pool.tile([SB, 1], i32, tag="rows", bufs=4)
                    nc.sync.dma_start(
                        out=rows[:, :], in_=rows_d[s * SB:(s + 1) * SB, :]
                    )
                    ent = ent_pool.tile([SB, EPAD], f32, tag="ent", bufs=4)
                    nc.gpsimd.memset(ent[:, H2:EPAD], 0.0)
                    nc.gpsimd.indirect_dma_start(
                        out=ent[:, 0:H2],
                        out_offset=None,
                        in_=hid_flat,
                        in_offset=bass.IndirectOffsetOnAxis(ap=rows[:, 0:1], axis=0),
                    )
                    ents.append(ent)
                return ents

            def entity_block(s, ents):
                """Latent-type + u2 for superbatch s (128 batches).
                Returns u2sb [50, 128] f32."""
                etT_list = []
                for ent in ents:
                    # transpose gathered entities -> entT [128, 5, 128] bf16
                    entT = ent_pool.tile([128, ECH, SB], bf16, tag="entT")
                    for c in range(ECH):
                        tp = ps_misc.tile([128, SB], f32, tag="misc")
                        nc.tensor.transpose(
                            tp[:, :], ent[:, c * 128:(c + 1) * 128], id_f32[:, :]
                        )
                        nc.vector.tensor_copy(entT[:, c, :], tp[:, :])
                    # latent-type logits: [3, 128] = sum_c ltT_c.T @ entT_c
                    lg_ps = ps_misc.tile([T, SB], f32, tag="misc")
                    for c in range(ECH):
                        nc.tensor.matmul(
                            lg_ps[:, :], lhsT=ltT[:, c, :], rhs=entT[:, c, :],
                            start=(c == 0), stop=(c == ECH - 1),
                        )
                    lgT_sb = ent_pool.tile([T, SB], f32, tag="lgT")
                    nc.vector.tensor_copy(lgT_sb[:, :], lg_ps[:, :])
                    lg2_ps = ps_misc.tile([SB, T], f32, tag="misc")
                    nc.tensor.transpose(lg2_ps[:, :], lgT_sb[:, :], id_f32[0:T, 0:T])
                    expl = ent_pool.tile([SB, T], f32, tag="expl")
                    nc.scalar.activation(expl[:, :], lg2_ps[:, :], AF.Exp)
                    ssum = ent_pool.tile([SB, 1], f32, tag="ssum")
                    nc.vector.reduce_sum(ssum[:, :], expl[:, :], axis=AX.X)
                    srec = ent_pool.tile([SB, 1], f32, tag="srec")
                    nc.vector.reciprocal(srec[:, :], ssum[:, :])
                    attw = ent_pool.tile([SB, T], f32, tag="attw")
                    nc.vector.tensor_scalar_mul(attw[:, :], expl[:, :], srec[:, 0:1])
                    awT_ps = ps_misc.tile([T, SB], f32, tag="misc")
                    nc.tensor.transpose(awT_ps[:, :], attw[:, :], id_f32[:, :])
                    awT = ent_pool.tile([T, SB], bf16, tag="awT_sb")
                    nc.vector.tensor_copy(awT[:, :], awT_ps[:, :])
                    # e_type = attw @ LT : [128, 600] (f32 psum, bf16 sbuf)
                    et_lo = ps_u1.tile([SB, 512], f32, tag="u1like")
                    et_hi = ps_misc.tile([SB, 128], f32, tag="misc")
                    nc.tensor.matmul(
                        et_lo[:, :], lhsT=awT[:, :], rhs=lt16[:, 0:512],
                        start=True, stop=True,
                    )
                    nc.tensor.matmul(
                        et_hi[:, 0:H2 - 512], lhsT=awT[:, :], rhs=lt16[:, 512:H2],
                        start=True, stop=True,
                    )
                    et = ent_pool.tile([SB, EPAD], bf16, tag="et_sb")
                    nc.gpsimd.memset(et[:, H2:EPAD], 0.0)
                    nc.scalar.activation(et[:, 0:512], et_lo[:, :], AF.Copy)
                    nc.scalar.activation(et[:, 512:H2], et_hi[:, 0:H2 - 512], AF.Copy)
                    # transpose e_type -> etT [128, 5, 128] bf16
                    etT = ent_pool.tile([128, ECH, SB], bf16, tag="etT")
                    etT_ps = ps_misc.tile([128, ECH, SB], bf16, tag="misc")
                    for c in range(ECH):
                        nc.tensor.transpose(
                            etT_ps[:, c, :], et[:, c * 128:(c + 1) * 128],
                            id_bf[:, :],
                        )
                    nc.vector.tensor_copy(etT[:, :, :], etT_ps[:, :, :])
                    etT_list.append((entT, etT))

                # u2T [50, 128] = sum over 20 chunks W_entT_c.T @ srcT_c
                order = [
                    etT_list[0][0], etT_list[0][1],
                    etT_list[1][0], etT_list[1][1],
                ]
                u2_ps = ps_misc.tile([A, SB], f32, tag="misc")
                k = 0
                for q in range(4):
                    for c in range(ECH):
                        nc.tensor.matmul(
                            u2_ps[:, :],
                            lhsT=wentT[:, q * ECH + c, :],
                            rhs=order[q][:, c, :],
                            start=(k == 0), stop=(k == 19),
                        )
                        k += 1
                u2sb = ent_pool.tile([A, SB], f32, tag="u2sb")
                nc.vector.tensor_copy(u2sb[:, :], u2_ps[:, :])
                return u2sb

            def emit_z(state):
                """z matmuls for a completed round (exp(scores) long since
                ready). Waves of 4 batches: each matmul uses a sliding
                32-col window of the UNNORMALIZED expT as stationary (col 0
                = this batch) against one batch's hp slab, placed at
                col-group 32q via tile_position; batch bl's row lands at
                partition 32q. A third 1-column matmul against ones gives
                the softmax denominator in the same partition, and the
                1/esum scale is folded into the PSUM evac."""
                if state is None or "z" in skip:
                    return
                b0, hp, expT, nbat, hoff = state
                zhi2 = None
                for w4 in range(nbat // 4):
                    zlo = ps_z.tile([128, 512], f32, tag="zlo")
                    if w4 % 2 == 0:
                        # two waves share one bank: doubles the recycling
                        # slack without spending another PSUM bank
                        zhi2 = ps_misc.tile([128, 2, 96], f32, tag="misc")
                    zhi = zhi2[:, w4 % 2, :]
                    for q in range(4):
                        bl = w4 * 4 + q
                        awin = expT[:, bl:bl + ROUND]
                        nc.tensor.matmul(
                            zlo[32 * q:32 * q + 32, :], lhsT=awin,
                            rhs=hp[:, hoff + bl, 0:512],
                            start=True, stop=True, tile_position=(0, 32 * q),
                        )
                        # skip_group_check: the sim's zero-region conflict
                        # tracker aliases the 4 col-group writes because
                        # zhi's 768 B/partition is below the 2 KiB region
                        # granularity; the writes hit disjoint partitions.
                        nc.tensor.matmul(
                            zhi[32 * q:32 * q + 32, 0:H2 - 512],
                            lhsT=awin,
                            rhs=hp[:, hoff + bl, 512:H2],
                            start=True, stop=True, tile_position=(0, 32 * q),
                            skip_group_check=True,
                        )
                        # esum[bl] into column 88 of the same partitions;
                        # start=False accumulates onto the pending-zero
                        # region left by the zhi matmul
                        nc.tensor.matmul(
                            zhi[32 * q:32 * q + 32, 88:89],
                            lhsT=awin,
                            rhs=ones_col[:, :],
                            start=False, stop=True, tile_position=(0, 32 * q),
                            skip_group_check=True,
                        )
                    erec = small.tile([128, 1], f32, tag="erec")
                    nc.vector.reciprocal(erec[:, :], zhi[:, 88:89])
                    z_sb = zs_pool.tile([128, H2], bf16, tag="z_sb")
                    nc.vector.tensor_scalar_mul(
                        z_sb[:, 0:512], zlo[:, :], erec[:, 0:1]
                    )
                    nc.vector.tensor_scalar_mul(
                        z_sb[:, 512:H2], zhi[:, 0:H2 - 512], erec[:, 0:1]
                    )
                    nc.sync.dma_start(
                        out=z_d[b0 + 4 * w4:b0 + 4 * w4 + 4, :],
                        in_=z_sb[0:128:32, :],
                    )

            def load_hp(b0):
                hp = hp_pool.tile([L, ROUND, F], bf16, tag="hp")
                # pos first (small), then hidden in 8-batch slices so the
                # first transposes only wait on the first slice
                nc.gpsimd.dma_start(
                    out=hp[:, :, H2:H2 + PP],
                    in_=pos1_d[b0:b0 + ROUND].rearrange("i l d -> l i d"),
                )
                nc.gpsimd.dma_start(
                    out=hp[:, :, H2 + PP:F],
                    in_=pos2_d[b0:b0 + ROUND].rearrange("i l d -> l i d"),
                )
                for k8 in range(4):
                    nc.gpsimd.dma_start(
                        out=hp[:, 8 * k8:8 * k8 + 8, 0:H2],
                        in_=hid_d[b0 + 8 * k8:b0 + 8 * k8 + 8].rearrange(
                            "i l d -> l i d"
                        ),
                    )
                return hp

            def do_round(s, r, u2sb, prev_state, last=False):
                b0 = s * SB + r * ROUND  # first batch of round (core-local)
                hp = load_hp(b0)

                # previous round's z goes first: its operands are ready, so
                # it fills the PE while this round's first transposes wait on
                # the hp DMAs, and it frees hp(r-1) early for the prefetcher
                emit_z(prev_state)

                sc_ps = ps_sc.tile([L, ROUND], f32, tag="scT")

                def emit_tp(g):
                    hT = ht_pool.tile(
                        [128, NCH // 2, 2, GROUP * L], fp8, tag="hT"
                    )
                    hTv = hT[:, :, :, :].rearrange("p cp o n -> p (cp o) n")
                    for j in range(GROUP):
                        bl = g * GROUP + j
                        tp = ps_tp.tile([128, NCH * L], bf16, tag="tp")
                        for c in range(NCH):
                            nc.tensor.transpose(
                                tp[:, c * L:(c + 1) * L],
                                hp[:, bl, CH_OFF[c]:CH_OFF[c] + 128],
                                id_bf[:, :],
                            )
                        tpv = tp[:, :].rearrange("p (c t) -> p c t", c=NCH)
                        nc.vector.tensor_copy(
                            hTv[:, 0:3, j * L:(j + 1) * L], tpv[:, 0:3, :]
                        )
                        nc.scalar.activation(
                            hTv[:, 3:NCH, j * L:(j + 1) * L], tpv[:, 3:NCH, :],
                            AF.Copy,
                        )
                    return hT

                def emit_u1(g, hT):
                    u1_ps = ps_u1.tile([64, GROUP * L], f32, tag="u1like")
                    for cp in range(NCH // 2):
                        nc.tensor.matmul(
                            u1_ps[:, :],
                            lhsT=whidT8[:, cp, :, :], rhs=hT[:, cp, :, :],
                            start=(cp == 0), stop=(cp == NCH // 2 - 1),
                            perf_mode=DR,
                        )
                    # tanh(u1 + u2) with the u2 column as per-partition bias
                    b0r = r * ROUND + g * GROUP
                    uT = u_pool.tile([A, GROUP * L], bf16, tag="uT")
                    for j in range(GROUP):
                        nc.scalar.activation(
                            uT[:, j * L:(j + 1) * L],
                            u1_ps[0:A, j * L:(j + 1) * L],
                            AF.Tanh,
                            bias=u2sb[:, b0r + j:b0r + j + 1],
                        )
                    return uT

                def emit_sc(g, uT):
                    for j in range(GROUP):
                        bl = g * GROUP + j
                        nc.tensor.matmul(
                            sc_ps[:, bl:bl + 1],
                            lhsT=uT[:, j * L:(j + 1) * L],
                            rhs=v16[:, 0:1],
                            start=True, stop=True,
                        )

                if "tp" in skip or "u1" in skip:
                    return None
                # staggered emission: transposes lead u1 by two groups and
                # the score matmuls trail by one, so the PE never sits on a
                # cross-engine latency (PSUM evac, tanh) with an empty queue
                H = ROUND // 2
                hTs = [None] * NG
                uTs = [None] * NG
                hTs[0] = emit_tp(0)
                for g in range(NG):
                    if g + 1 < NG:
                        hTs[g + 1] = emit_tp(g + 1)
                    uTs[g] = emit_u1(g, hTs[g])
                    hTs[g] = None
                    if g >= 1:
                        emit_sc(g - 1, uTs[g - 1])
                        uTs[g - 1] = None
                    if last and g in (3, 5, 7) and "sm" not in skip:
                        # last round: softmax+z in 8-batch quarters emitted
                        # as soon as their scores exist, so only the final
                        # quarter runs after everything else
                        Q = (g - 3) // 2
                        expQ = small.tile(
                            [L, 8 + ROUND - 1], bf16, tag="alphaT"
                        )
                        nc.vector.memset(expQ[:, 8:], 0.0)
                        nc.scalar.activation(
                            expQ[:, 0:8], sc_ps[:, 8 * Q:8 * Q + 8], AF.Exp
                        )
                        emit_z((b0 + 8 * Q, hp, expQ, 8, 8 * Q))
                emit_sc(NG - 1, uTs[NG - 1])

                if "sm" in skip:
                    return None
                # deferred-normalization softmax: just exponentiate in the
                # token-major layout; the denominator and 1/esum scale are
                # computed inside emit_z. Cols past the batch count are a
                # zero pad so a sliding 32-wide window expT[:, bl:bl+32] is
                # always in bounds.
                if not last:
                    expT = small.tile([L, 2 * ROUND - 1], bf16, tag="alphaT")
                    nc.vector.memset(expT[:, ROUND:], 0.0)
                    nc.scalar.activation(expT[:, 0:ROUND], sc_ps[:, :], AF.Exp)
                    return (b0, hp, expT, ROUND, 0)
                # last round: quarters 0-2 were handled mid-round; only the
                # final 8 batches remain for the tail
                expB = small.tile([L, 8 + ROUND - 1], bf16, tag="alphaT")
                nc.vector.memset(expB[:, 8:], 0.0)
                nc.scalar.activation(expB[:, 0:8], sc_ps[:, 24:ROUND], AF.Exp)
                return (b0 + 24, hp, expB, 8, 24)

            # both entity blocks run up front: they only read DRAM inputs,
            # and their PE work fills the initial hidden-DMA wait
            u2sbs = []
            if "ent" in skip:
                for s in range(BC // SB):
                    u2sb_t = ent_pool.tile([A, SB], f32, tag="u2sb")
                    nc.gpsimd.memset(u2sb_t[:, :], 0.0)
                    u2sbs.append(u2sb_t)
            else:
                all_ents = [entity_gather(s) for s in range(BC // SB)]
                for s in range(BC // SB):
                    u2sbs.append(entity_block(s, all_ents[s]))
            state = None
            nsb, nr = BC // SB, SB // ROUND
            for s in range(nsb):
                for r in range(nr):
                    last = (s == nsb - 1 and r == nr - 1)
                    state = do_round(s, r, u2sbs[s], state, last=last)
            emit_z(state)

    nc.compile()
    return nc


def _get_nc():
    if "nc" not in _CACHE:
        _CACHE["nc"] = _build_bass()
    return _CACHE["nc"]


def make_in_maps(inputs):
    hidden = np.ascontiguousarray(np.asarray(inputs["hidden"], dtype=np.float32))
    pos1 = np.ascontiguousarray(np.asarray(inputs["pos1_emb"], dtype=np.float32))
    pos2 = np.ascontiguousarray(np.asarray(inputs["pos2_emb"], dtype=np.float32))
    e1 = np.asarray(inputs["entity1_idx"]).astype(np.int64)
    e2 = np.asarray(inputs["entity2_idx"]).astype(np.int64)
    w_hid = np.ascontiguousarray(np.asarray(inputs["W_hid"], dtype=np.float32))
    w_ent = np.ascontiguousarray(np.asarray(inputs["W_ent"], dtype=np.float32))
    lt = np.ascontiguousarray(np.asarray(inputs["latent_types"], dtype=np.float32))
    v = np.ascontiguousarray(np.asarray(inputs["v"], dtype=np.float32))

    loc = np.arange(BC, dtype=np.int64) * L
    in_maps = []
    for c in range(NCORES):
        sl = slice(c * BC, (c + 1) * BC)
        in_maps.append({
            "hidden": hidden[sl],
            "pos1": pos1[sl],
            "pos2": pos2[sl],
            "e1rows": np.ascontiguousarray(
                (loc + e1[sl]).astype(np.int32)[:, None]),
            "e2rows": np.ascontiguousarray(
                (loc + e2[sl]).astype(np.int32)[:, None]),
            "w_hid": w_hid,
            "w_ent": w_ent,
            "latent": lt,
            "v": v,
        })
    return in_maps


def unshard_z(zt):
    return np.asarray(zt, dtype=np.float32)


def kernel(**inputs):
    from concourse.bass_utils import run_bass_kernel_spmd

    nc = _get_nc()
    in_maps = make_in_maps(inputs)
    res = run_bass_kernel_spmd(nc, in_maps, core_ids=list(range(NCORES)))
    _CACHE["last_res"] = res
    outs = [unshard_z(r["z"]) for r in res.results]
    return np.concatenate(outs, axis=0).astype(np.float32)
